# revision 12
# baseline (speedup 1.0000x reference)
"""Trainium2 Bass kernel for nn_Attention (B=16, C=8, H=W=512).

Per sample b:
  q = Wq.x + bq   [1,H,W]
  k = Wk.x + bk   [1,H,W]
  v = Wv.x + bv   [C,H,W]
  S[i,j] = sum_w q[i,w] k[j,w]; A = softmax_j(S); out[c,i,w] = sum_j A[i,j] v[c,j,w]

Sharding: data-parallel over batch, 2 samples per core, 8 cores, no collectives.

Per-core dataflow (per sample):
  - x loaded in "grouped" layout xg[r]: [128=(g,c), 512=w], g=16 rows, c=8 channels.
  - 1x1 convs as block-diagonal matmuls on TensorE (f32r):
      v-MM: lhsT=Wv_bd [128,128] -> psum grouped v; qk-MM: lhsT=Wqk_bd [128,32],
      4 col-group MMs fill a [128,512] psum = 64 rows of interleaved q/k.
  - v: psum -> SBUF (bias add + bf16 cast on ScalarE), then SBUF->SBUF DMA
      rearranges grouped v into channel-plane tiles vplane[jt]: [128=j, 8=c, 512=w].
  - q,k: psum -> SBUF (bias add), PE-transpose (f32r) -> qkT[wt]: [128=w, 1024=(q|k cols)].
  - S-MM (f32r): per i-tile accumulate 4 w-tiles; softmax via DVE reduce_max +
      ScalarE Exp(bias=-max, accum_out=rowsum) -> E (bf16, unnormalized).
  - A-transpose (bf16) -> AT[jt]; out-MM (bf16): per (i-tile, c) accumulate 4 j-tiles.
  - out: psum -> SBUF scaled by 1/rowsum (per-partition) on DVE, DMA out per i-tile.
"""

import os
import sys

import numpy as np

B, C, H, W = 16, 8, 512, 512
NCORES = 8
BPC = B // NCORES  # samples per core
P = 128
G = 16  # rows per group (P // C)
NR = H // G  # 32 row-groups per sample

_CACHE = {}


def _build():
    if "nc" in _CACHE:
        return _CACHE["nc"]
    sys.path.insert(0, "/opt/trn_rl_repo")
    import concourse.bass as bass
    import concourse.tile as tile
    from concourse import bacc, mybir

    f32 = mybir.dt.float32
    f32r = mybir.dt.float32r
    bf16 = mybir.dt.bfloat16
    AF = mybir.ActivationFunctionType
    AX = mybir.AxisListType

    nc = bacc.Bacc("TRN2", target_bir_lowering=False, debug=False)

    xh_d = nc.declare_dram_parameter("xh", [BPC, C, H, W], bf16, isOutput=False)
    xl_d = nc.declare_dram_parameter("xl", [BPC, C, H, W], bf16, isOutput=False)
    wqkh_d = nc.declare_dram_parameter("wqkh", [P, 32], bf16, isOutput=False)
    wqkl_d = nc.declare_dram_parameter("wqkl", [P, 32], bf16, isOutput=False)
    wv_d = nc.declare_dram_parameter("wv", [P, P], bf16, isOutput=False)
    bqk_d = nc.declare_dram_parameter("bqk", [P, 1], f32, isOutput=False)
    bv_d = nc.declare_dram_parameter("bv", [P, 1], f32, isOutput=False)
    idf_d = nc.declare_dram_parameter("identf", [P, P], f32, isOutput=False)
    idb_d = nc.declare_dram_parameter("identb", [P, P], bf16, isOutput=False)
    out_d = nc.declare_dram_parameter("out", [BPC, C, H, W], f32, isOutput=True)

    with tile.TileContext(nc) as tc:
        with (
            tc.tile_pool(name="consts", bufs=1) as consts,
            tc.tile_pool(name="xg", bufs=6) as xg_pool,
            tc.tile_pool(name="vg", bufs=4) as vg_pool,
            tc.tile_pool(name="vplane", bufs=2) as vp_pool,
            tc.tile_pool(name="qksb", bufs=10) as qk_pool,
            tc.tile_pool(name="qkt", bufs=2) as qkt_pool,
            tc.tile_pool(name="esb", bufs=8) as e_pool,
            tc.tile_pool(name="atsb", bufs=8) as at_pool,
            tc.tile_pool(name="osb", bufs=2) as o_pool,
            tc.tile_pool(name="stats", bufs=16) as st_pool,
            tc.tile_pool(name="ps", bufs=7, space="PSUM") as ps_pool,
            tc.tile_pool(name="ps_at", bufs=1, space="PSUM") as psat_pool,
        ):
            wqkh = consts.tile([P, 32], bf16)
            nc.sync.dma_start(wqkh[:], wqkh_d.ap())
            wqkl = consts.tile([P, 32], bf16)
            nc.sync.dma_start(wqkl[:], wqkl_d.ap())
            wv = consts.tile([P, P], bf16)
            nc.sync.dma_start(wv[:], wv_d.ap())
            bqk = consts.tile([P, 1], f32)
            nc.sync.dma_start(bqk[:], bqk_d.ap())
            bv = consts.tile([P, 1], f32)
            nc.sync.dma_start(bv[:], bv_d.ap())
            idf = consts.tile([P, P], f32)
            nc.sync.dma_start(idf[:], idf_d.ap())
            idb = consts.tile([P, P], bf16)
            nc.sync.dma_start(idb[:], idb_d.ap())

            for b in range(BPC):
                # DRAM views of x[b] by row-group: [r, g, c, w] (g,c = partition dims)
                xbh = xh_d.ap()[b].rearrange("c (r g) w -> r g c w", g=G)
                xbl = xl_d.ap()[b].rearrange("c (r g) w -> r g c w", g=G)
                # ---- phase 1: stream row-groups; conv matmuls ----
                vplane = [vp_pool.tile([P, C, W], bf16, name=f"vplane{i}") for i in range(4)]
                qk_psums = []
                qk_sb = []
                for r in range(NR):
                    xgh = xg_pool.tile([P, W], bf16, name="xgh")
                    nc.sync.dma_start(xgh[:], xbh[r])
                    xgl = xg_pool.tile([P, W], bf16, name="xgl")
                    nc.sync.dma_start(xgl[:], xbl[r])
                    # v conv: grouped psum (hi only; bf16 v is accurate enough)
                    psv = ps_pool.tile([P, W], f32, name="ps")
                    nc.tensor.matmul(psv[:], wv[:], xgh[:], start=True, stop=True)
                    vg = vg_pool.tile([P, W], bf16)
                    nc.scalar.activation(vg[:], psv[:], AF.Identity, bias=bv[:])
                    # bridge: grouped -> channel-plane (SBUF->SBUF DMA)
                    jt, sl = r // 8, G * (r % 8)
                    nc.sync.dma_start(vplane[jt][sl : sl + G, :, :], vg[:])
                    # qk conv: 4 col-group MMs per [128,512] psum (64 rows)
                    if r % 4 == 0:
                        psqk = ps_pool.tile([P, W], f32, name="ps")
                        qk_psums.append(psqk)
                    m = r % 4
                    for wpart, xgpart, st, sp in (
                        (wqkh, xgh, True, False),
                        (wqkh, xgl, False, False),
                        (wqkl, xgh, False, True),
                    ):
                        nc.tensor.matmul(
                            qk_psums[-1][32 * m : 32 * m + 32, :],
                            wpart[:],
                            xgpart[:],
                            start=st,
                            stop=sp,
                            tile_position=(0, 32 * m),
                            skip_group_check=True,
                        )
                    if m == 3:
                        t = r // 4
                        sb = qk_pool.tile([P, W], f32)
                        nc.scalar.activation(sb[:], qk_psums[-1][:], AF.Identity, bias=bqk[:])
                        qk_sb.append((t, sb))

                # ---- phase 2: transpose q/k -> qkT[wt] [128=w, 1024=(64 blocks)] ----
                qkt = [qkt_pool.tile([P, 2 * W], f32, name=f"qkt{i}") for i in range(4)]
                for wt in range(4):
                    for th in range(2):
                        pst = ps_pool.tile([P, W], f32, name="ps")
                        for q in range(4):
                            t, sb = qk_sb[4 * th + q]
                            nc.tensor.transpose(
                                pst[:, P * q : P * q + P],
                                sb[:, P * wt : P * wt + P],
                                idf[:],
                            )
                        # de-interleave: psum free = [blk(4) x pair(4) x qk(2) x s(16)]
                        # -> qkt free = [qk(2) x 256] at offsets (256*th, 512+256*th)
                        dst = qkt[wt][:].rearrange(
                            "p (qk half blk pair s) -> p qk half blk pair s",
                            qk=2, half=2, pair=4, s=G,
                        )[:, :, th]
                        nc.scalar.copy(
                            dst,
                            pst[:].rearrange(
                                "p (blk pair qk s) -> p qk blk pair s",
                                qk=2, s=G, pair=4,
                            ),
                        )

                # ---- phase 3: S matmul + softmax ----
                e_sb = []
                rs_all = []
                for it in range(4):
                    pss = ps_pool.tile([P, W], f32, name="ps")
                    for wt in range(4):
                        nc.tensor.matmul(
                            pss[:],
                            qkt[wt][:, P * it : P * it + P],
                            qkt[wt][:, W:],
                            start=(wt == 0),
                            stop=(wt == 3),
                        )
                    mx = st_pool.tile([P, 1], f32)
                    nc.vector.reduce_max(mx[:], pss[:], axis=AX.X, negate=True)
                    esb = e_pool.tile([P, W], bf16)
                    sm = st_pool.tile([P, 1], f32)
                    nc.scalar.activation(
                        esb[:], pss[:], AF.Exp, bias=mx[:], accum_out=sm[:]
                    )
                    rs = st_pool.tile([P, 1], f32)
                    nc.vector.reciprocal(rs[:], sm[:])
                    e_sb.append(esb)
                    rs_all.append(rs)

                # ---- phase 4: transpose A (bf16) -> AT[jt] ----
                at_sb = []
                for jt in range(4):
                    psa = psat_pool.tile([P, W], bf16, name="psa")
                    for it in range(4):
                        nc.tensor.transpose(
                            psa[:, P * it : P * it + P],
                            e_sb[it][:, P * jt : P * jt + P],
                            idb[:],
                        )
                    atsb = at_pool.tile([P, W], bf16)
                    nc.vector.tensor_copy(atsb[:], psa[:])
                    at_sb.append(atsb)

                # ---- phase 5: out matmul + normalize + store ----
                for it in range(4):
                    osb = o_pool.tile([P, C, W], f32)
                    for c in range(C):
                        pso = ps_pool.tile([P, W], f32, name="ps")
                        for jt in range(4):
                            nc.tensor.matmul(
                                pso[:],
                                at_sb[jt][:, P * it : P * it + P],
                                vplane[jt][:, c, :],
                                start=(jt == 0),
                                stop=(jt == 3),
                            )
                        nc.vector.tensor_scalar_mul(osb[:, c, :], pso[:], rs_all[it][:])
                    nc.sync.dma_start(
                        out_d.ap()[b, :, P * it : P * it + P, :].rearrange(
                            "c i w -> i c w"
                        ),
                        osb[:],
                    )

    nc.compile()
    _CACHE["nc"] = nc
    return nc


def _make_consts(Wq, bq, Wk, bk, Wv, bv):
    wqk = np.zeros((P, 32), np.float32)
    for g in range(G):
        for c in range(C):
            wqk[g * C + c, g] = Wq[0, c]
            wqk[g * C + c, 16 + g] = Wk[0, c]
    wv = np.zeros((P, P), np.float32)
    for g in range(G):
        for ci in range(C):
            for co in range(C):
                wv[g * C + ci, g * C + co] = Wv[co, ci]
    bqk = np.concatenate([np.full(16, bq[0]), np.full(16, bk[0])] * 4).astype(
        np.float32
    )[:, None]
    bvv = np.tile(bv.astype(np.float32), G)[:, None]
    import ml_dtypes

    eyef = np.eye(P, dtype=np.float32)
    eyeb = np.eye(P).astype(ml_dtypes.bfloat16)
    wqkh = wqk.astype(ml_dtypes.bfloat16)
    wqkl = (wqk - wqkh.astype(np.float32)).astype(ml_dtypes.bfloat16)
    return (wqkh, wqkl, wv.astype(ml_dtypes.bfloat16), bqk, bvv, eyef, eyeb)


def kernel(x, Wq, bq, Wk, bk, Wv, bv):
    sys.path.insert(0, "/opt/trn_rl_repo")
    from concourse.bass_utils import run_bass_kernel_spmd

    nc = _build()
    wqkh, wqkl, wv, bqk, bvv, eyef, eyeb = _make_consts(
        np.asarray(Wq), np.asarray(bq), np.asarray(Wk), np.asarray(bk),
        np.asarray(Wv), np.asarray(bv),
    )
    import ml_dtypes

    x = np.asarray(x, dtype=np.float32)
    xh = x.astype(ml_dtypes.bfloat16)
    xl = (x - xh.astype(np.float32)).astype(ml_dtypes.bfloat16)
    xh = np.ascontiguousarray(xh)
    xl = np.ascontiguousarray(xl)
    in_maps = []
    for core in range(NCORES):
        in_maps.append(
            {
                "xh": xh[BPC * core : BPC * core + BPC],
                "xl": xl[BPC * core : BPC * core + BPC],
                "wqkh": wqkh,
                "wqkl": wqkl,
                "wv": wv,
                "bqk": bqk,
                "bv": bvv,
                "identf": eyef,
                "identb": eyeb,
            }
        )
    res = run_bass_kernel_spmd(nc, in_maps, core_ids=list(range(NCORES)))
    out = np.concatenate([r["out"] for r in res.results], axis=0)
    return out


# revision 14
# speedup vs baseline: 1.3847x; 1.3847x over previous
"""Trainium2 Bass kernel for nn_Attention (B=16, C=8, H=W=512).

Per sample b:
  q = Wq.x + bq   [1,H,W]
  k = Wk.x + bk   [1,H,W]
  v = Wv.x + bv   [C,H,W]
  S[i,j] = sum_w q[i,w] k[j,w]; A = softmax_j(S); out[c,i,w] = sum_j A[i,j] v[c,j,w]

Sharding: data-parallel over batch, 2 samples per core, 8 cores, no collectives.

Per-core dataflow (per sample):
  - x loaded in "grouped" layout xg[r]: [128=(g,c), 512=w], g=16 rows, c=8 channels.
  - 1x1 convs as block-diagonal matmuls on TensorE (f32r):
      v-MM: lhsT=Wv_bd [128,128] -> psum grouped v; qk-MM: lhsT=Wqk_bd [128,32],
      4 col-group MMs fill a [128,512] psum = 64 rows of interleaved q/k.
  - v: psum -> SBUF (bias add + bf16 cast on ScalarE), then SBUF->SBUF DMA
      rearranges grouped v into channel-plane tiles vplane[jt]: [128=j, 8=c, 512=w].
  - q,k: psum -> SBUF (bias add), PE-transpose (f32r) -> qkT[wt]: [128=w, 1024=(q|k cols)].
  - S-MM (f32r): per i-tile accumulate 4 w-tiles; softmax via DVE reduce_max +
      ScalarE Exp(bias=-max, accum_out=rowsum) -> E (bf16, unnormalized).
  - A-transpose (bf16) -> AT[jt]; out-MM (bf16): per (i-tile, c) accumulate 4 j-tiles.
  - out: psum -> SBUF scaled by 1/rowsum (per-partition) on DVE, DMA out per i-tile.
"""

import os
import sys

import numpy as np

B, C, H, W = 16, 8, 512, 512
NCORES = 8
BPC = B // NCORES  # samples per core
P = 128
G = 16  # rows per group (P // C)
NR = H // G  # 32 row-groups per sample

_CACHE = {}


def _build():
    if "nc" in _CACHE:
        return _CACHE["nc"]
    sys.path.insert(0, "/opt/trn_rl_repo")
    import concourse.bass as bass
    import concourse.tile as tile
    from concourse import bacc, mybir

    f32 = mybir.dt.float32
    f32r = mybir.dt.float32r
    bf16 = mybir.dt.bfloat16
    AF = mybir.ActivationFunctionType
    AX = mybir.AxisListType

    nc = bacc.Bacc("TRN2", target_bir_lowering=False, debug=False)

    xh_d = nc.declare_dram_parameter("xh", [BPC, P, NR, W], bf16, isOutput=False)
    xl_d = nc.declare_dram_parameter("xl", [BPC, P, NR, W], bf16, isOutput=False)
    wqkh_d = nc.declare_dram_parameter("wqkh", [P, 32], bf16, isOutput=False)
    wqkl_d = nc.declare_dram_parameter("wqkl", [P, 32], bf16, isOutput=False)
    wv_d = nc.declare_dram_parameter("wv", [P, P], bf16, isOutput=False)
    bqk_d = nc.declare_dram_parameter("bqk", [P, 1], f32, isOutput=False)
    bv_d = nc.declare_dram_parameter("bv", [P, 1], f32, isOutput=False)
    idf_d = nc.declare_dram_parameter("identf", [P, P], f32, isOutput=False)
    idb_d = nc.declare_dram_parameter("identb", [P, P], bf16, isOutput=False)
    out_d = nc.declare_dram_parameter("out", [BPC, C, H, W], f32, isOutput=True)

    with tile.TileContext(nc) as tc:
        with (
            tc.tile_pool(name="consts", bufs=1) as consts,
            tc.tile_pool(name="xg", bufs=2) as xg_pool,
            tc.tile_pool(name="vg", bufs=4) as vg_pool,
            tc.tile_pool(name="vplane", bufs=2) as vp_pool,
            tc.tile_pool(name="qksb", bufs=10) as qk_pool,
            tc.tile_pool(name="qkt", bufs=2) as qkt_pool,
            tc.tile_pool(name="esb", bufs=8) as e_pool,
            tc.tile_pool(name="atsb", bufs=8) as at_pool,
            tc.tile_pool(name="osb", bufs=2) as o_pool,
            tc.tile_pool(name="stats", bufs=16) as st_pool,
            tc.tile_pool(name="ps", bufs=7, space="PSUM") as ps_pool,
            tc.tile_pool(name="ps_at", bufs=1, space="PSUM") as psat_pool,
        ):
            wqkh = consts.tile([P, 32], bf16)
            nc.sync.dma_start(wqkh[:], wqkh_d.ap())
            wqkl = consts.tile([P, 32], bf16)
            nc.sync.dma_start(wqkl[:], wqkl_d.ap())
            wv = consts.tile([P, P], bf16)
            nc.sync.dma_start(wv[:], wv_d.ap())
            bqk = consts.tile([P, 1], f32)
            nc.sync.dma_start(bqk[:], bqk_d.ap())
            bv = consts.tile([P, 1], f32)
            nc.sync.dma_start(bv[:], bv_d.ap())
            idf = consts.tile([P, P], f32)
            nc.sync.dma_start(idf[:], idf_d.ap())
            idb = consts.tile([P, P], bf16)
            nc.sync.dma_start(idb[:], idb_d.ap())

            for b in range(BPC):
                xbh = xh_d.ap()[b]  # [128, NR, W] grouped-partition-major
                xbl = xl_d.ap()[b]
                # ---- phase 1: stream row-groups; conv matmuls ----
                vplane = [vp_pool.tile([P, C, W], bf16, name=f"vplane{i}") for i in range(4)]
                qk_psums = []
                qk_sb = []
                RB = 8  # row-groups per x DMA (8KB contiguous runs)
                for r in range(NR):
                    if r % RB == 0:
                        xqh = xg_pool.tile([P, RB, W], bf16, name="xqh")
                        nc.sync.dma_start(xqh[:], xbh[:, r : r + RB, :])
                        xql = xg_pool.tile([P, RB, W], bf16, name="xql")
                        nc.sync.dma_start(xql[:], xbl[:, r : r + RB, :])
                    xgh = xqh[:, r % RB, :]
                    xgl = xql[:, r % RB, :]
                    # v conv: grouped psum (hi only; bf16 v is accurate enough)
                    psv = ps_pool.tile([P, W], f32, name="ps")
                    nc.tensor.matmul(psv[:], wv[:], xgh, start=True, stop=True)
                    vg = vg_pool.tile([P, W], bf16)
                    nc.scalar.activation(vg[:], psv[:], AF.Identity, bias=bv[:])
                    # bridge: grouped -> channel-plane (SBUF->SBUF DMA)
                    jt, sl = r // 8, G * (r % 8)
                    nc.scalar.dma_start(vplane[jt][sl : sl + G, :, :], vg[:])
                    # qk conv: 4 col-group MMs per [128,512] psum (64 rows)
                    if r % 4 == 0:
                        psqk = ps_pool.tile([P, W], f32, name="ps")
                        qk_psums.append(psqk)
                    m = r % 4
                    for wpart, xgpart, st, sp in (
                        (wqkh, xgh, True, False),
                        (wqkh, xgl, False, False),
                        (wqkl, xgh, False, True),
                    ):
                        nc.tensor.matmul(
                            qk_psums[-1][32 * m : 32 * m + 32, :],
                            wpart[:],
                            xgpart,
                            start=st,
                            stop=sp,
                            tile_position=(0, 32 * m),
                            skip_group_check=True,
                        )
                    if m == 3:
                        t = r // 4
                        sb = qk_pool.tile([P, W], f32)
                        nc.scalar.activation(sb[:], qk_psums[-1][:], AF.Identity, bias=bqk[:])
                        qk_sb.append((t, sb))

                # ---- phase 2: transpose q/k -> qkT[wt] [128=w, 1024=(64 blocks)] ----
                qkt = [qkt_pool.tile([P, 2 * W], f32, name=f"qkt{i}") for i in range(4)]
                for wt in range(4):
                    for th in range(2):
                        pst = ps_pool.tile([P, W], f32, name="ps")
                        for q in range(4):
                            t, sb = qk_sb[4 * th + q]
                            nc.tensor.transpose(
                                pst[:, P * q : P * q + P],
                                sb[:, P * wt : P * wt + P],
                                idf[:],
                            )
                        # de-interleave: psum free = [blk(4) x pair(4) x qk(2) x s(16)]
                        # -> qkt free = [qk(2) x 256] at offsets (256*th, 512+256*th)
                        dst = qkt[wt][:].rearrange(
                            "p (qk half blk pair s) -> p qk half blk pair s",
                            qk=2, half=2, pair=4, s=G,
                        )[:, :, th]
                        nc.scalar.copy(
                            dst,
                            pst[:].rearrange(
                                "p (blk pair qk s) -> p qk blk pair s",
                                qk=2, s=G, pair=4,
                            ),
                        )

                # ---- phase 3: S matmul + softmax ----
                e_sb = []
                rs_all = []
                for it in range(4):
                    pss = ps_pool.tile([P, W], f32, name="ps")
                    for wt in range(4):
                        nc.tensor.matmul(
                            pss[:],
                            qkt[wt][:, P * it : P * it + P],
                            qkt[wt][:, W:],
                            start=(wt == 0),
                            stop=(wt == 3),
                        )
                    mx = st_pool.tile([P, 1], f32)
                    nc.vector.reduce_max(mx[:], pss[:], axis=AX.X, negate=True)
                    esb = e_pool.tile([P, W], bf16)
                    sm = st_pool.tile([P, 1], f32)
                    nc.scalar.activation(
                        esb[:], pss[:], AF.Exp, bias=mx[:], accum_out=sm[:]
                    )
                    rs = st_pool.tile([P, 1], f32)
                    nc.vector.reciprocal(rs[:], sm[:])
                    e_sb.append(esb)
                    rs_all.append(rs)

                # ---- phase 4: transpose A (bf16) -> AT[jt] ----
                at_sb = []
                for jt in range(4):
                    psa = psat_pool.tile([P, W], bf16, name="psa")
                    for it in range(4):
                        nc.tensor.transpose(
                            psa[:, P * it : P * it + P],
                            e_sb[it][:, P * jt : P * jt + P],
                            idb[:],
                        )
                    atsb = at_pool.tile([P, W], bf16)
                    nc.vector.tensor_copy(atsb[:], psa[:])
                    at_sb.append(atsb)

                # ---- phase 5: out matmul + normalize + store ----
                for it in range(4):
                    for ch in range(2):
                        osb = o_pool.tile([P, 4, W], f32)
                        for cc in range(4):
                            c = 4 * ch + cc
                            pso = ps_pool.tile([P, W], f32, name="ps")
                            for jt in range(4):
                                nc.tensor.matmul(
                                    pso[:],
                                    at_sb[jt][:, P * it : P * it + P],
                                    vplane[jt][:, c, :],
                                    start=(jt == 0),
                                    stop=(jt == 3),
                                )
                            nc.vector.tensor_scalar_mul(
                                osb[:, cc, :], pso[:], rs_all[it][:]
                            )
                        nc.scalar.dma_start(
                            out_d.ap()[
                                b, 4 * ch : 4 * ch + 4, P * it : P * it + P, :
                            ].rearrange("c i w -> i c w"),
                            osb[:],
                        )

    nc.compile()
    _CACHE["nc"] = nc
    return nc


def _make_consts(Wq, bq, Wk, bk, Wv, bv):
    wqk = np.zeros((P, 32), np.float32)
    for g in range(G):
        for c in range(C):
            wqk[g * C + c, g] = Wq[0, c]
            wqk[g * C + c, 16 + g] = Wk[0, c]
    wv = np.zeros((P, P), np.float32)
    for g in range(G):
        for ci in range(C):
            for co in range(C):
                wv[g * C + ci, g * C + co] = Wv[co, ci]
    bqk = np.concatenate([np.full(16, bq[0]), np.full(16, bk[0])] * 4).astype(
        np.float32
    )[:, None]
    bvv = np.tile(bv.astype(np.float32), G)[:, None]
    import ml_dtypes

    eyef = np.eye(P, dtype=np.float32)
    eyeb = np.eye(P).astype(ml_dtypes.bfloat16)
    wqkh = wqk.astype(ml_dtypes.bfloat16)
    wqkl = (wqk - wqkh.astype(np.float32)).astype(ml_dtypes.bfloat16)
    return (wqkh, wqkl, wv.astype(ml_dtypes.bfloat16), bqk, bvv, eyef, eyeb)


def _split_x(x):
    import ml_dtypes

    x = np.asarray(x, dtype=np.float32)
    xh = x.astype(ml_dtypes.bfloat16)
    xl = (x - xh.astype(np.float32)).astype(ml_dtypes.bfloat16)
    # [B,C,H,W] -> [B, (g c)=128, r=NR, W]   (p = g*C + c, i = r*G + g)
    perm = lambda a: np.ascontiguousarray(
        a.reshape(B, C, NR, G, W).transpose(0, 3, 1, 2, 4).reshape(B, G * C, NR, W)
    )
    return perm(xh), perm(xl)


def kernel(x, Wq, bq, Wk, bk, Wv, bv):
    sys.path.insert(0, "/opt/trn_rl_repo")
    from concourse.bass_utils import run_bass_kernel_spmd

    nc = _build()
    wqkh, wqkl, wv, bqk, bvv, eyef, eyeb = _make_consts(
        np.asarray(Wq), np.asarray(bq), np.asarray(Wk), np.asarray(bk),
        np.asarray(Wv), np.asarray(bv),
    )
    xh, xl = _split_x(x)
    in_maps = []
    for core in range(NCORES):
        in_maps.append(
            {
                "xh": xh[BPC * core : BPC * core + BPC],
                "xl": xl[BPC * core : BPC * core + BPC],
                "wqkh": wqkh,
                "wqkl": wqkl,
                "wv": wv,
                "bqk": bqk,
                "bv": bvv,
                "identf": eyef,
                "identb": eyeb,
            }
        )
    res = run_bass_kernel_spmd(nc, in_maps, core_ids=list(range(NCORES)))
    out = np.concatenate([r["out"] for r in res.results], axis=0)
    return out


# revision 15
# speedup vs baseline: 1.4839x; 1.0716x over previous
"""Trainium2 Bass kernel for nn_Attention (B=16, C=8, H=W=512).

Per sample b:
  q = Wq.x + bq   [1,H,W]
  k = Wk.x + bk   [1,H,W]
  v = Wv.x + bv   [C,H,W]
  S[i,j] = sum_w q[i,w] k[j,w]; A = softmax_j(S); out[c,i,w] = sum_j A[i,j] v[c,j,w]

Sharding: data-parallel over batch, 2 samples per core, 8 cores, no collectives.

Per-core dataflow (per sample):
  - x loaded in "grouped" layout xg[r]: [128=(g,c), 512=w], g=16 rows, c=8 channels.
  - 1x1 convs as block-diagonal matmuls on TensorE (f32r):
      v-MM: lhsT=Wv_bd [128,128] -> psum grouped v; qk-MM: lhsT=Wqk_bd [128,32],
      4 col-group MMs fill a [128,512] psum = 64 rows of interleaved q/k.
  - v: psum -> SBUF (bias add + bf16 cast on ScalarE), then SBUF->SBUF DMA
      rearranges grouped v into channel-plane tiles vplane[jt]: [128=j, 8=c, 512=w].
  - q,k: psum -> SBUF (bias add), PE-transpose (f32r) -> qkT[wt]: [128=w, 1024=(q|k cols)].
  - S-MM (f32r): per i-tile accumulate 4 w-tiles; softmax via DVE reduce_max +
      ScalarE Exp(bias=-max, accum_out=rowsum) -> E (bf16, unnormalized).
  - A-transpose (bf16) -> AT[jt]; out-MM (bf16): per (i-tile, c) accumulate 4 j-tiles.
  - out: psum -> SBUF scaled by 1/rowsum (per-partition) on DVE, DMA out per i-tile.
"""

import os
import sys

import numpy as np

B, C, H, W = 16, 8, 512, 512
NCORES = 8
BPC = B // NCORES  # samples per core
P = 128
G = 16  # rows per group (P // C)
NR = H // G  # 32 row-groups per sample

_CACHE = {}


def _build():
    if "nc" in _CACHE:
        return _CACHE["nc"]
    sys.path.insert(0, "/opt/trn_rl_repo")
    import concourse.bass as bass
    import concourse.tile as tile
    from concourse import bacc, mybir

    f32 = mybir.dt.float32
    f32r = mybir.dt.float32r
    bf16 = mybir.dt.bfloat16
    AF = mybir.ActivationFunctionType
    AX = mybir.AxisListType

    nc = bacc.Bacc("TRN2", target_bir_lowering=False, debug=False)

    xh_d = nc.declare_dram_parameter("xh", [BPC, P, NR, W], bf16, isOutput=False)
    xl_d = nc.declare_dram_parameter("xl", [BPC, P, NR, W], bf16, isOutput=False)
    wqkh_d = nc.declare_dram_parameter("wqkh", [P, 32], bf16, isOutput=False)
    wqkl_d = nc.declare_dram_parameter("wqkl", [P, 32], bf16, isOutput=False)
    wv_d = nc.declare_dram_parameter("wv", [P, P], bf16, isOutput=False)
    bqk_d = nc.declare_dram_parameter("bqk", [P, 1], f32, isOutput=False)
    bv_d = nc.declare_dram_parameter("bv", [P, 1], f32, isOutput=False)
    idf_d = nc.declare_dram_parameter("identf", [P, P], f32, isOutput=False)
    idb_d = nc.declare_dram_parameter("identb", [P, P], bf16, isOutput=False)
    out_d = nc.declare_dram_parameter("out", [BPC, C, H, W], f32, isOutput=True)

    with tile.TileContext(nc) as tc:
        with (
            tc.tile_pool(name="consts", bufs=1) as consts,
            tc.tile_pool(name="xg", bufs=2) as xg_pool,
            tc.tile_pool(name="vg", bufs=4) as vg_pool,
            tc.tile_pool(name="vplane", bufs=2) as vp_pool,
            tc.tile_pool(name="qksb", bufs=10) as qk_pool,
            tc.tile_pool(name="qkt", bufs=2) as qkt_pool,
            tc.tile_pool(name="esb", bufs=8) as e_pool,
            tc.tile_pool(name="atsb", bufs=8) as at_pool,
            tc.tile_pool(name="osb", bufs=2) as o_pool,
            tc.tile_pool(name="stats", bufs=16) as st_pool,
            tc.tile_pool(name="ps", bufs=7, space="PSUM") as ps_pool,
            tc.tile_pool(name="ps_at", bufs=1, space="PSUM") as psat_pool,
        ):
            wqkh = consts.tile([P, 32], bf16)
            nc.sync.dma_start(wqkh[:], wqkh_d.ap())
            wqkl = consts.tile([P, 32], bf16)
            nc.sync.dma_start(wqkl[:], wqkl_d.ap())
            wv = consts.tile([P, P], bf16)
            nc.sync.dma_start(wv[:], wv_d.ap())
            bqk = consts.tile([P, 1], f32)
            nc.sync.dma_start(bqk[:], bqk_d.ap())
            bv = consts.tile([P, 1], f32)
            nc.sync.dma_start(bv[:], bv_d.ap())
            idf = consts.tile([P, P], f32)
            nc.sync.dma_start(idf[:], idf_d.ap())
            idb = consts.tile([P, P], bf16)
            nc.sync.dma_start(idb[:], idb_d.ap())

            for b in range(BPC):
                xbh = xh_d.ap()[b]  # [128, NR, W] grouped-partition-major
                xbl = xl_d.ap()[b]
                # ---- phase 1: stream row-groups; conv matmuls ----
                vplane = [vp_pool.tile([P, C, W], bf16, name=f"vplane{i}") for i in range(4)]
                qk_psums = []
                qk_sb = []
                RB = 8  # row-groups per x DMA (8KB contiguous runs)
                for r in range(NR):
                    if r % RB == 0:
                        xqh = xg_pool.tile([P, RB, W], bf16, name="xqh")
                        nc.sync.dma_start(xqh[:], xbh[:, r : r + RB, :])
                        xql = xg_pool.tile([P, RB, W], bf16, name="xql")
                        nc.sync.dma_start(xql[:], xbl[:, r : r + RB, :])
                    xgh = xqh[:, r % RB, :]
                    xgl = xql[:, r % RB, :]
                    # v conv: grouped psum (hi only; bf16 v is accurate enough)
                    psv = ps_pool.tile([P, W], f32, name="ps")
                    nc.tensor.matmul(psv[:], wv[:], xgh, start=True, stop=True)
                    vg = vg_pool.tile([P, W], bf16)
                    nc.scalar.activation(vg[:], psv[:], AF.Identity, bias=bv[:])
                    # bridge: grouped -> channel-plane (SBUF->SBUF DMA)
                    jt, sl = r // 8, G * (r % 8)
                    nc.scalar.dma_start(vplane[jt][sl : sl + G, :, :], vg[:])
                    # qk conv: 4 col-group MMs per [128,512] psum (64 rows)
                    if r % 4 == 0:
                        psqk = ps_pool.tile([P, W], f32, name="ps")
                        qk_psums.append(psqk)
                    m = r % 4
                    for wpart, xgpart, st, sp in (
                        (wqkh, xgh, True, False),
                        (wqkh, xgl, False, False),
                        (wqkl, xgh, False, True),
                    ):
                        nc.tensor.matmul(
                            qk_psums[-1][32 * m : 32 * m + 32, :],
                            wpart[:],
                            xgpart,
                            start=st,
                            stop=sp,
                            tile_position=(0, 32 * m),
                            skip_group_check=True,
                        )
                    if m == 3:
                        t = r // 4
                        sb = qk_pool.tile([P, W], f32)
                        nc.scalar.activation(sb[:], qk_psums[-1][:], AF.Identity, bias=bqk[:])
                        qk_sb.append((t, sb))

                # ---- phase 2: transpose q/k -> qkT[wt] [128=w, 1024=(64 blocks)] ----
                qkt = [qkt_pool.tile([P, 2 * W], f32r, name=f"qkt{i}") for i in range(4)]
                for wt in range(4):
                    for th in range(2):
                        pst = ps_pool.tile([P, W], f32, name="ps")
                        for q in range(4):
                            t, sb = qk_sb[4 * th + q]
                            nc.tensor.transpose(
                                pst[:, P * q : P * q + P],
                                sb[:, P * wt : P * wt + P],
                                idf[:],
                            )
                        # de-interleave: psum free = [blk(4) x pair(4) x qk(2) x s(16)]
                        # -> qkt free = [qk(2) x 256] at offsets (256*th, 512+256*th)
                        dst = qkt[wt][:].rearrange(
                            "p (qk half blk pair s) -> p qk half blk pair s",
                            qk=2, half=2, pair=4, s=G,
                        )[:, :, th]
                        nc.scalar.copy(
                            dst,
                            pst[:].rearrange(
                                "p (blk pair qk s) -> p qk blk pair s",
                                qk=2, s=G, pair=4,
                            ),
                        )

                # ---- phase 3: S matmul + softmax ----
                e_sb = []
                rs_all = []
                for it in range(4):
                    pss = ps_pool.tile([P, W], f32, name="ps")
                    for wt in range(4):
                        nc.tensor.matmul(
                            pss[:],
                            qkt[wt][:, P * it : P * it + P],
                            qkt[wt][:, W:],
                            start=(wt == 0),
                            stop=(wt == 3),
                        )
                    mx = st_pool.tile([P, 1], f32)
                    nc.vector.reduce_max(mx[:], pss[:], axis=AX.X, negate=True)
                    esb = e_pool.tile([P, W], bf16)
                    sm = st_pool.tile([P, 1], f32)
                    nc.scalar.activation(
                        esb[:], pss[:], AF.Exp, bias=mx[:], accum_out=sm[:]
                    )
                    rs = st_pool.tile([P, 1], f32)
                    nc.vector.reciprocal(rs[:], sm[:])
                    e_sb.append(esb)
                    rs_all.append(rs)

                # ---- phase 4: transpose A (bf16) -> AT[jt] ----
                at_sb = []
                for jt in range(4):
                    psa = psat_pool.tile([P, W], bf16, name="psa")
                    for it in range(4):
                        nc.tensor.transpose(
                            psa[:, P * it : P * it + P],
                            e_sb[it][:, P * jt : P * jt + P],
                            idb[:],
                        )
                    atsb = at_pool.tile([P, W], bf16)
                    nc.vector.tensor_copy(atsb[:], psa[:])
                    at_sb.append(atsb)

                # ---- phase 5: out matmul + normalize + store ----
                for it in range(4):
                    for ch in range(2):
                        osb = o_pool.tile([P, 4, W], f32)
                        for cc in range(4):
                            c = 4 * ch + cc
                            pso = ps_pool.tile([P, W], f32, name="ps")
                            for jt in range(4):
                                nc.tensor.matmul(
                                    pso[:],
                                    at_sb[jt][:, P * it : P * it + P],
                                    vplane[jt][:, c, :],
                                    start=(jt == 0),
                                    stop=(jt == 3),
                                )
                            nc.vector.tensor_scalar_mul(
                                osb[:, cc, :], pso[:], rs_all[it][:]
                            )
                        nc.scalar.dma_start(
                            out_d.ap()[
                                b, 4 * ch : 4 * ch + 4, P * it : P * it + P, :
                            ].rearrange("c i w -> i c w"),
                            osb[:],
                        )

    nc.compile()
    _CACHE["nc"] = nc
    return nc


def _make_consts(Wq, bq, Wk, bk, Wv, bv):
    wqk = np.zeros((P, 32), np.float32)
    for g in range(G):
        for c in range(C):
            wqk[g * C + c, g] = Wq[0, c]
            wqk[g * C + c, 16 + g] = Wk[0, c]
    wv = np.zeros((P, P), np.float32)
    for g in range(G):
        for ci in range(C):
            for co in range(C):
                wv[g * C + ci, g * C + co] = Wv[co, ci]
    bqk = np.concatenate([np.full(16, bq[0]), np.full(16, bk[0])] * 4).astype(
        np.float32
    )[:, None]
    bvv = np.tile(bv.astype(np.float32), G)[:, None]
    import ml_dtypes

    eyef = np.eye(P, dtype=np.float32)
    eyeb = np.eye(P).astype(ml_dtypes.bfloat16)
    wqkh = wqk.astype(ml_dtypes.bfloat16)
    wqkl = (wqk - wqkh.astype(np.float32)).astype(ml_dtypes.bfloat16)
    return (wqkh, wqkl, wv.astype(ml_dtypes.bfloat16), bqk, bvv, eyef, eyeb)


def _split_x(x):
    import ml_dtypes

    x = np.asarray(x, dtype=np.float32)
    xh = x.astype(ml_dtypes.bfloat16)
    xl = (x - xh.astype(np.float32)).astype(ml_dtypes.bfloat16)
    # [B,C,H,W] -> [B, (g c)=128, r=NR, W]   (p = g*C + c, i = r*G + g)
    perm = lambda a: np.ascontiguousarray(
        a.reshape(B, C, NR, G, W).transpose(0, 3, 1, 2, 4).reshape(B, G * C, NR, W)
    )
    return perm(xh), perm(xl)


def kernel(x, Wq, bq, Wk, bk, Wv, bv):
    sys.path.insert(0, "/opt/trn_rl_repo")
    from concourse.bass_utils import run_bass_kernel_spmd

    nc = _build()
    wqkh, wqkl, wv, bqk, bvv, eyef, eyeb = _make_consts(
        np.asarray(Wq), np.asarray(bq), np.asarray(Wk), np.asarray(bk),
        np.asarray(Wv), np.asarray(bv),
    )
    xh, xl = _split_x(x)
    in_maps = []
    for core in range(NCORES):
        in_maps.append(
            {
                "xh": xh[BPC * core : BPC * core + BPC],
                "xl": xl[BPC * core : BPC * core + BPC],
                "wqkh": wqkh,
                "wqkl": wqkl,
                "wv": wv,
                "bqk": bqk,
                "bv": bvv,
                "identf": eyef,
                "identb": eyeb,
            }
        )
    res = run_bass_kernel_spmd(nc, in_maps, core_ids=list(range(NCORES)))
    out = np.concatenate([r["out"] for r in res.results], axis=0)
    return out


# revision 20
# speedup vs baseline: 1.6209x; 1.0923x over previous
"""Trainium2 Bass kernel for nn_Attention (B=16, C=8, H=W=512).

Per sample b:
  q = Wq.x + bq   [1,H,W]
  k = Wk.x + bk   [1,H,W]
  v = Wv.x + bv   [C,H,W]
  S[i,j] = sum_w q[i,w] k[j,w]; A = softmax_j(S); out[c,i,w] = sum_j A[i,j] v[c,j,w]

Sharding: data-parallel over batch, 2 samples per core, 8 cores, no collectives.

Per-core dataflow (per sample):
  - x loaded in "grouped" layout xg[r]: [128=(g,c), 512=w], g=16 rows, c=8 channels.
  - 1x1 convs as block-diagonal matmuls on TensorE (f32r):
      v-MM: lhsT=Wv_bd [128,128] -> psum grouped v; qk-MM: lhsT=Wqk_bd [128,32],
      4 col-group MMs fill a [128,512] psum = 64 rows of interleaved q/k.
  - v: psum -> SBUF (bias add + bf16 cast on ScalarE), then SBUF->SBUF DMA
      rearranges grouped v into channel-plane tiles vplane[jt]: [128=j, 8=c, 512=w].
  - q,k: psum -> SBUF (bias add), PE-transpose (f32r) -> qkT[wt]: [128=w, 1024=(q|k cols)].
  - S-MM (f32r): per i-tile accumulate 4 w-tiles; softmax via DVE reduce_max +
      ScalarE Exp(bias=-max, accum_out=rowsum) -> E (bf16, unnormalized).
  - A-transpose (bf16) -> AT[jt]; out-MM (bf16): per (i-tile, c) accumulate 4 j-tiles.
  - out: psum -> SBUF scaled by 1/rowsum (per-partition) on DVE, DMA out per i-tile.
"""

import os
import sys

import numpy as np

B, C, H, W = 16, 8, 512, 512
NCORES = 8
BPC = B // NCORES  # samples per core
P = 128
G = 16  # rows per group (P // C)
NR = H // G  # 32 row-groups per sample

_CACHE = {}


def _build():
    if "nc" in _CACHE:
        return _CACHE["nc"]
    sys.path.insert(0, "/opt/trn_rl_repo")
    import concourse.bass as bass
    import concourse.tile as tile
    from concourse import bacc, mybir

    f32 = mybir.dt.float32
    f32r = mybir.dt.float32r
    bf16 = mybir.dt.bfloat16
    AF = mybir.ActivationFunctionType
    AX = mybir.AxisListType

    nc = bacc.Bacc("TRN2", target_bir_lowering=False, debug=False)

    xh_d = nc.declare_dram_parameter("xh", [BPC, P, NR, W], bf16, isOutput=False)
    xl_d = nc.declare_dram_parameter("xl", [BPC, P, NR, W], bf16, isOutput=False)
    wqkh_d = nc.declare_dram_parameter("wqkh", [P, 32], bf16, isOutput=False)
    wqkl_d = nc.declare_dram_parameter("wqkl", [P, 32], bf16, isOutput=False)
    wv_d = nc.declare_dram_parameter("wv", [P, P], bf16, isOutput=False)
    bqk_d = nc.declare_dram_parameter("bqk", [P, 1], f32, isOutput=False)
    bv_d = nc.declare_dram_parameter("bv", [P, 1], f32, isOutput=False)
    idf_d = nc.declare_dram_parameter("identf", [P, P], f32, isOutput=False)
    idb_d = nc.declare_dram_parameter("identb", [P, P], bf16, isOutput=False)
    out_d = nc.declare_dram_parameter("out", [BPC, C, H, W], f32, isOutput=True)

    with tile.TileContext(nc) as tc:
        with (
            tc.tile_pool(name="consts", bufs=1) as consts,
            tc.tile_pool(name="xg", bufs=2) as xg_pool,
            tc.tile_pool(name="vg", bufs=4) as vg_pool,
            tc.tile_pool(name="vplane", bufs=2) as vp_pool,
            tc.tile_pool(name="qksb", bufs=10) as qk_pool,
            tc.tile_pool(name="qkt", bufs=2) as qkt_pool,
            tc.tile_pool(name="esb", bufs=8) as e_pool,
            tc.tile_pool(name="atsb", bufs=8) as at_pool,
            tc.tile_pool(name="osb", bufs=2) as o_pool,
            tc.tile_pool(name="stats", bufs=16) as st_pool,
            tc.tile_pool(name="ps", bufs=3, space="PSUM") as ps_pool,
            tc.tile_pool(name="ps_v", bufs=1, space="PSUM") as psv_pool,
            tc.tile_pool(name="ps_at", bufs=1, space="PSUM") as psat_pool,
        ):
            wqkh = consts.tile([P, 32], bf16)
            nc.sync.dma_start(wqkh[:], wqkh_d.ap())
            wqkl = consts.tile([P, 32], bf16)
            nc.sync.dma_start(wqkl[:], wqkl_d.ap())
            wv = consts.tile([P, P], bf16)
            nc.sync.dma_start(wv[:], wv_d.ap())
            bqk = consts.tile([P, 1], f32)
            nc.sync.dma_start(bqk[:], bqk_d.ap())
            bv = consts.tile([P, 1], f32)
            nc.sync.dma_start(bv[:], bv_d.ap())
            idf = consts.tile([P, P], f32)
            nc.sync.dma_start(idf[:], idf_d.ap())
            idb = consts.tile([P, P], bf16)
            nc.sync.dma_start(idb[:], idb_d.ap())

            for b in range(BPC):
                xbh = xh_d.ap()[b]  # [128, NR, W] grouped-partition-major
                xbl = xl_d.ap()[b]
                # ---- phase 1: stream row-groups; conv matmuls ----
                vplane = [vp_pool.tile([P, C, W], bf16, name=f"vplane{i}") for i in range(4)]
                qk_psums = []
                qk_sb = []
                RB = 8  # row-groups per x DMA (8KB contiguous runs)
                for r in range(NR):
                    if r % RB == 0:
                        xqh = xg_pool.tile([P, RB, W], bf16, name="xqh")
                        nc.sync.dma_start(xqh[:], xbh[:, r : r + RB, :])
                        xql = xg_pool.tile([P, RB, W], bf16, name="xql")
                        nc.sync.dma_start(xql[:], xbl[:, r : r + RB, :])
                    xgh = xqh[:, r % RB, :]
                    xgl = xql[:, r % RB, :]
                    # v conv: grouped psum, batched 4 row-groups per 4-bank psum
                    if r % 4 == 0:
                        psv = psv_pool.tile([P, 4, W], f32, name="psv")
                    nc.tensor.matmul(
                        psv[:, r % 4, :], wv[:], xgh, start=True, stop=True
                    )
                    if r % 4 == 3:
                        vg = vg_pool.tile([P, 4, W], bf16)
                        nc.scalar.activation(
                            vg[:], psv[:], AF.Identity, bias=bv[:]
                        )
                        # bridge: grouped -> channel-plane (SBUF->SBUF DMA)
                        for ri in range(4):
                            rr = r - 3 + ri
                            jt, sl = rr // 8, G * (rr % 8)
                            nc.sync.dma_start(
                                vplane[jt][sl : sl + G, :, :], vg[:, ri, :]
                            )
                    # qk conv: 4 col-group MMs per [128,512] psum (64 rows)
                    if r % 4 == 0:
                        psqk = ps_pool.tile([P, W], f32, name="ps")
                        qk_psums.append(psqk)
                    m = r % 4
                    for wpart, xgpart, st, sp in (
                        (wqkh, xgh, True, False),
                        (wqkh, xgl, False, False),
                        (wqkl, xgh, False, True),
                    ):
                        nc.tensor.matmul(
                            qk_psums[-1][32 * m : 32 * m + 32, :],
                            wpart[:],
                            xgpart,
                            start=st,
                            stop=sp,
                            tile_position=(0, 32 * m),
                            skip_group_check=True,
                        )
                    if m == 3:
                        t = r // 4
                        sb = qk_pool.tile([P, W], f32)
                        nc.scalar.activation(sb[:], qk_psums[-1][:], AF.Identity, bias=bqk[:])
                        qk_sb.append((t, sb))

                # ---- phase 2: transpose q/k -> qkT[wt] [128=w, 1024=(64 blocks)] ----
                qkt = [qkt_pool.tile([P, 2 * W], f32r, name=f"qkt{i}") for i in range(4)]
                for wt in range(4):
                    for th in range(2):
                        pst = ps_pool.tile([P, W], f32, name="ps")
                        for q in range(4):
                            t, sb = qk_sb[4 * th + q]
                            nc.tensor.transpose(
                                pst[:, P * q : P * q + P],
                                sb[:, P * wt : P * wt + P],
                                idf[:],
                            )
                        # de-interleave: psum free = [blk(4) x pair(4) x qk(2) x s(16)]
                        # -> qkt free = [qk(2) x 256] at offsets (256*th, 512+256*th)
                        dst = qkt[wt][:].rearrange(
                            "p (qk half blk pair s) -> p qk half blk pair s",
                            qk=2, half=2, pair=4, s=G,
                        )[:, :, th]
                        nc.scalar.copy(
                            dst,
                            pst[:].rearrange(
                                "p (blk pair qk s) -> p qk blk pair s",
                                qk=2, s=G, pair=4,
                            ),
                        )

                # ---- phase 3: S matmul + softmax ----
                e_sb = []
                rs_all = []
                for it in range(4):
                    pss = ps_pool.tile([P, W], f32, name="ps")
                    for wt in range(4):
                        nc.tensor.matmul(
                            pss[:],
                            qkt[wt][:, P * it : P * it + P],
                            qkt[wt][:, W:],
                            start=(wt == 0),
                            stop=(wt == 3),
                        )
                    mx = st_pool.tile([P, 1], f32)
                    nc.vector.reduce_max(mx[:], pss[:], axis=AX.X, negate=True)
                    esb = e_pool.tile([P, W], bf16)
                    sm = st_pool.tile([P, 1], f32)
                    nc.scalar.activation(
                        esb[:], pss[:], AF.Exp, bias=mx[:], accum_out=sm[:]
                    )
                    rs = st_pool.tile([P, 1], f32)
                    nc.vector.reciprocal(rs[:], sm[:])
                    e_sb.append(esb)
                    rs_all.append(rs)

                # ---- phase 4: transpose A (bf16) -> AT[jt] ----
                at_sb = []
                for jt in range(4):
                    psa = psat_pool.tile([P, W], bf16, name="psa")
                    for it in range(4):
                        nc.tensor.transpose(
                            psa[:, P * it : P * it + P],
                            e_sb[it][:, P * jt : P * jt + P],
                            idb[:],
                        )
                    atsb = at_pool.tile([P, W], bf16)
                    nc.vector.tensor_copy(atsb[:], psa[:])
                    at_sb.append(atsb)

                # ---- phase 5: out matmul + normalize + store ----
                for it in range(4):
                    for ch in range(2):
                        osb = o_pool.tile([P, 4, W], f32)
                        for cc in range(4):
                            c = 4 * ch + cc
                            pso = ps_pool.tile([P, W], f32, name="ps")
                            for jt in range(4):
                                nc.tensor.matmul(
                                    pso[:],
                                    at_sb[jt][:, P * it : P * it + P],
                                    vplane[jt][:, c, :],
                                    start=(jt == 0),
                                    stop=(jt == 3),
                                )
                            nc.vector.tensor_scalar_mul(
                                osb[:, cc, :], pso[:], rs_all[it][:]
                            )
                        nc.scalar.dma_start(
                            out_d.ap()[
                                b, 4 * ch : 4 * ch + 4, P * it : P * it + P, :
                            ].rearrange("c i w -> i c w"),
                            osb[:],
                        )

    nc.compile()
    _CACHE["nc"] = nc
    return nc


def _make_consts(Wq, bq, Wk, bk, Wv, bv):
    wqk = np.zeros((P, 32), np.float32)
    for g in range(G):
        for c in range(C):
            wqk[g * C + c, g] = Wq[0, c]
            wqk[g * C + c, 16 + g] = Wk[0, c]
    wv = np.zeros((P, P), np.float32)
    for g in range(G):
        for ci in range(C):
            for co in range(C):
                wv[g * C + ci, g * C + co] = Wv[co, ci]
    bqk = np.concatenate([np.full(16, bq[0]), np.full(16, bk[0])] * 4).astype(
        np.float32
    )[:, None]
    bvv = np.tile(bv.astype(np.float32), G)[:, None]
    import ml_dtypes

    eyef = np.eye(P, dtype=np.float32)
    eyeb = np.eye(P).astype(ml_dtypes.bfloat16)
    wqkh = wqk.astype(ml_dtypes.bfloat16)
    wqkl = (wqk - wqkh.astype(np.float32)).astype(ml_dtypes.bfloat16)
    return (wqkh, wqkl, wv.astype(ml_dtypes.bfloat16), bqk, bvv, eyef, eyeb)


def _split_x(x):
    import ml_dtypes

    x = np.asarray(x, dtype=np.float32)
    xh = x.astype(ml_dtypes.bfloat16)
    xl = (x - xh.astype(np.float32)).astype(ml_dtypes.bfloat16)
    # [B,C,H,W] -> [B, (g c)=128, r=NR, W]   (p = g*C + c, i = r*G + g)
    perm = lambda a: np.ascontiguousarray(
        a.reshape(B, C, NR, G, W).transpose(0, 3, 1, 2, 4).reshape(B, G * C, NR, W)
    )
    return perm(xh), perm(xl)


def kernel(x, Wq, bq, Wk, bk, Wv, bv):
    sys.path.insert(0, "/opt/trn_rl_repo")
    from concourse.bass_utils import run_bass_kernel_spmd

    nc = _build()
    wqkh, wqkl, wv, bqk, bvv, eyef, eyeb = _make_consts(
        np.asarray(Wq), np.asarray(bq), np.asarray(Wk), np.asarray(bk),
        np.asarray(Wv), np.asarray(bv),
    )
    xh, xl = _split_x(x)
    in_maps = []
    for core in range(NCORES):
        in_maps.append(
            {
                "xh": xh[BPC * core : BPC * core + BPC],
                "xl": xl[BPC * core : BPC * core + BPC],
                "wqkh": wqkh,
                "wqkl": wqkl,
                "wv": wv,
                "bqk": bqk,
                "bv": bvv,
                "identf": eyef,
                "identb": eyeb,
            }
        )
    res = run_bass_kernel_spmd(nc, in_maps, core_ids=list(range(NCORES)))
    out = np.concatenate([r["out"] for r in res.results], axis=0)
    return out


# revision 21
# speedup vs baseline: 1.6845x; 1.0393x over previous
"""Trainium2 Bass kernel for nn_Attention (B=16, C=8, H=W=512).

Per sample b:
  q = Wq.x + bq   [1,H,W]
  k = Wk.x + bk   [1,H,W]
  v = Wv.x + bv   [C,H,W]
  S[i,j] = sum_w q[i,w] k[j,w]; A = softmax_j(S); out[c,i,w] = sum_j A[i,j] v[c,j,w]

Sharding: data-parallel over batch, 2 samples per core, 8 cores, no collectives.

Per-core dataflow (per sample):
  - x loaded in "grouped" layout xg[r]: [128=(g,c), 512=w], g=16 rows, c=8 channels.
  - 1x1 convs as block-diagonal matmuls on TensorE (f32r):
      v-MM: lhsT=Wv_bd [128,128] -> psum grouped v; qk-MM: lhsT=Wqk_bd [128,32],
      4 col-group MMs fill a [128,512] psum = 64 rows of interleaved q/k.
  - v: psum -> SBUF (bias add + bf16 cast on ScalarE), then SBUF->SBUF DMA
      rearranges grouped v into channel-plane tiles vplane[jt]: [128=j, 8=c, 512=w].
  - q,k: psum -> SBUF (bias add), PE-transpose (f32r) -> qkT[wt]: [128=w, 1024=(q|k cols)].
  - S-MM (f32r): per i-tile accumulate 4 w-tiles; softmax via DVE reduce_max +
      ScalarE Exp(bias=-max, accum_out=rowsum) -> E (bf16, unnormalized).
  - A-transpose (bf16) -> AT[jt]; out-MM (bf16): per (i-tile, c) accumulate 4 j-tiles.
  - out: psum -> SBUF scaled by 1/rowsum (per-partition) on DVE, DMA out per i-tile.
"""

import os
import sys

import numpy as np

B, C, H, W = 16, 8, 512, 512
NCORES = 8
BPC = B // NCORES  # samples per core
P = 128
G = 16  # rows per group (P // C)
NR = H // G  # 32 row-groups per sample

_CACHE = {}


def _build():
    if "nc" in _CACHE:
        return _CACHE["nc"]
    sys.path.insert(0, "/opt/trn_rl_repo")
    import concourse.bass as bass
    import concourse.tile as tile
    from concourse import bacc, mybir

    f32 = mybir.dt.float32
    f32r = mybir.dt.float32r
    bf16 = mybir.dt.bfloat16
    AF = mybir.ActivationFunctionType
    AX = mybir.AxisListType

    nc = bacc.Bacc("TRN2", target_bir_lowering=False, debug=False)

    xh_d = nc.declare_dram_parameter("xh", [BPC, P, NR, W], bf16, isOutput=False)
    xl_d = nc.declare_dram_parameter("xl", [BPC, P, NR, W], bf16, isOutput=False)
    wqkh_d = nc.declare_dram_parameter("wqkh", [P, 32], bf16, isOutput=False)
    wqkl_d = nc.declare_dram_parameter("wqkl", [P, 32], bf16, isOutput=False)
    wv_d = nc.declare_dram_parameter("wv", [P, P], bf16, isOutput=False)
    bqk_d = nc.declare_dram_parameter("bqk", [P, 1], f32, isOutput=False)
    bv_d = nc.declare_dram_parameter("bv", [P, 1], f32, isOutput=False)
    idf_d = nc.declare_dram_parameter("identf", [P, P], f32, isOutput=False)
    idb_d = nc.declare_dram_parameter("identb", [P, P], bf16, isOutput=False)
    out_d = nc.declare_dram_parameter("out", [BPC, C, H, W], f32, isOutput=True)

    with tile.TileContext(nc) as tc:
        with (
            tc.tile_pool(name="consts", bufs=1) as consts,
            tc.tile_pool(name="xg", bufs=2) as xg_pool,
            tc.tile_pool(name="vg", bufs=4) as vg_pool,
            tc.tile_pool(name="vplane", bufs=2) as vp_pool,
            tc.tile_pool(name="qksb", bufs=10) as qk_pool,
            tc.tile_pool(name="qkt", bufs=2) as qkt_pool,
            tc.tile_pool(name="esb", bufs=8) as e_pool,
            tc.tile_pool(name="atsb", bufs=8) as at_pool,
            tc.tile_pool(name="osb", bufs=2) as o_pool,
            tc.tile_pool(name="stats", bufs=16) as st_pool,
            tc.tile_pool(name="ps", bufs=3, space="PSUM") as ps_pool,
            tc.tile_pool(name="ps_v", bufs=1, space="PSUM") as psv_pool,
            tc.tile_pool(name="ps_at", bufs=1, space="PSUM") as psat_pool,
        ):
            wqkh = consts.tile([P, 32], bf16)
            nc.sync.dma_start(wqkh[:], wqkh_d.ap())
            wqkl = consts.tile([P, 32], bf16)
            nc.sync.dma_start(wqkl[:], wqkl_d.ap())
            wv = consts.tile([P, P], bf16)
            nc.sync.dma_start(wv[:], wv_d.ap())
            bqk = consts.tile([P, 1], f32)
            nc.sync.dma_start(bqk[:], bqk_d.ap())
            bv = consts.tile([P, 1], f32)
            nc.sync.dma_start(bv[:], bv_d.ap())
            idf = consts.tile([P, P], f32)
            nc.sync.dma_start(idf[:], idf_d.ap())
            idb = consts.tile([P, P], bf16)
            nc.sync.dma_start(idb[:], idb_d.ap())

            for b in range(BPC):
                xbh = xh_d.ap()[b]  # [128, NR, W] grouped-partition-major
                xbl = xl_d.ap()[b]
                # ---- phase 1: stream row-groups; conv matmuls ----
                vplane = [vp_pool.tile([P, C, W], bf16, name=f"vplane{i}") for i in range(4)]
                qk_psums = []
                qk_sb = []
                RB = 8  # row-groups per x DMA (8KB contiguous runs)
                for r in range(NR):
                    if r % RB == 0:
                        xqh = xg_pool.tile([P, RB, W], bf16, name="xqh")
                        nc.sync.dma_start(xqh[:], xbh[:, r : r + RB, :])
                        xql = xg_pool.tile([P, RB, W], bf16, name="xql")
                        nc.sync.dma_start(xql[:], xbl[:, r : r + RB, :])
                    xgh = xqh[:, r % RB, :]
                    xgl = xql[:, r % RB, :]
                    # v conv: grouped psum, batched 4 row-groups per 4-bank psum
                    if r % 4 == 0:
                        psv = psv_pool.tile([P, 4, W], f32, name="psv")
                    nc.tensor.matmul(
                        psv[:, r % 4, :], wv[:], xgh, start=True, stop=True
                    )
                    if r % 4 == 3:
                        vg = vg_pool.tile([P, 4, W], bf16)
                        nc.scalar.activation(
                            vg[:], psv[:], AF.Identity, bias=bv[:]
                        )
                        # bridge: grouped -> channel-plane (SBUF->SBUF DMA)
                        for ri in range(4):
                            rr = r - 3 + ri
                            jt, sl = rr // 8, G * (rr % 8)
                            eng = nc.sync if ri % 2 == 0 else nc.scalar
                            eng.dma_start(
                                vplane[jt][sl : sl + G, :, :], vg[:, ri, :]
                            )
                    # qk conv: 4 col-group MMs per [128,512] psum (64 rows)
                    if r % 4 == 0:
                        psqk = ps_pool.tile([P, W], f32, name="ps")
                        qk_psums.append(psqk)
                    m = r % 4
                    for wpart, xgpart, st, sp in (
                        (wqkh, xgh, True, False),
                        (wqkh, xgl, False, False),
                        (wqkl, xgh, False, True),
                    ):
                        nc.tensor.matmul(
                            qk_psums[-1][32 * m : 32 * m + 32, :],
                            wpart[:],
                            xgpart,
                            start=st,
                            stop=sp,
                            tile_position=(0, 32 * m),
                            skip_group_check=True,
                        )
                    if m == 3:
                        t = r // 4
                        sb = qk_pool.tile([P, W], f32)
                        nc.scalar.activation(sb[:], qk_psums[-1][:], AF.Identity, bias=bqk[:])
                        qk_sb.append((t, sb))

                # ---- phase 2: transpose q/k -> qkT[wt] [128=w, 1024=(64 blocks)] ----
                qkt = [qkt_pool.tile([P, 2 * W], f32r, name=f"qkt{i}") for i in range(4)]
                for wt in range(4):
                    for th in range(2):
                        pst = ps_pool.tile([P, W], f32, name="ps")
                        for q in range(4):
                            t, sb = qk_sb[4 * th + q]
                            nc.tensor.transpose(
                                pst[:, P * q : P * q + P],
                                sb[:, P * wt : P * wt + P],
                                idf[:],
                            )
                        # de-interleave: psum free = [blk(4) x pair(4) x qk(2) x s(16)]
                        # -> qkt free = [qk(2) x 256] at offsets (256*th, 512+256*th)
                        dst = qkt[wt][:].rearrange(
                            "p (qk half blk pair s) -> p qk half blk pair s",
                            qk=2, half=2, pair=4, s=G,
                        )[:, :, th]
                        ceng = nc.scalar if th == 0 else nc.vector
                        csrc = pst[:].rearrange(
                            "p (blk pair qk s) -> p qk blk pair s",
                            qk=2, s=G, pair=4,
                        )
                        if th == 0:
                            nc.scalar.copy(dst, csrc)
                        else:
                            nc.vector.tensor_copy(dst, csrc)

                # ---- phase 3: S matmul + softmax ----
                e_sb = []
                rs_all = []
                for it in range(4):
                    pss = ps_pool.tile([P, W], f32, name="ps")
                    qoff = 256 * (it // 2) + P * (it % 2)
                    for jh in range(2):
                        for wt in range(4):
                            nc.tensor.matmul(
                                pss[:, 256 * jh : 256 * jh + 256],
                                qkt[wt][:, qoff : qoff + P],
                                qkt[wt][:, W + 256 * jh : W + 256 * jh + 256],
                                start=(wt == 0),
                                stop=(wt == 3),
                                skip_group_check=True,
                            )
                    mx = st_pool.tile([P, 1], f32)
                    nc.vector.reduce_max(mx[:], pss[:], axis=AX.X, negate=True)
                    esb = e_pool.tile([P, W], bf16)
                    sm = st_pool.tile([P, 1], f32)
                    nc.scalar.activation(
                        esb[:], pss[:], AF.Exp, bias=mx[:], accum_out=sm[:]
                    )
                    rs = st_pool.tile([P, 1], f32)
                    nc.vector.reciprocal(rs[:], sm[:])
                    e_sb.append(esb)
                    rs_all.append(rs)

                # ---- phase 4: transpose A (bf16) -> AT[jt] ----
                at_sb = []
                for jt in range(4):
                    psa = psat_pool.tile([P, W], bf16, name="psa")
                    for it in range(4):
                        nc.tensor.transpose(
                            psa[:, P * it : P * it + P],
                            e_sb[it][:, P * jt : P * jt + P],
                            idb[:],
                        )
                    atsb = at_pool.tile([P, W], bf16)
                    nc.vector.tensor_copy(atsb[:], psa[:])
                    at_sb.append(atsb)

                # ---- phase 5: out matmul + normalize + store ----
                for it in range(4):
                    for ch in range(2):
                        osb = o_pool.tile([P, 4, W], f32)
                        for cc in range(4):
                            c = 4 * ch + cc
                            pso = ps_pool.tile([P, W], f32, name="ps")
                            for jt in range(4):
                                nc.tensor.matmul(
                                    pso[:],
                                    at_sb[jt][:, P * it : P * it + P],
                                    vplane[jt][:, c, :],
                                    start=(jt == 0),
                                    stop=(jt == 3),
                                )
                            nc.vector.tensor_scalar_mul(
                                osb[:, cc, :], pso[:], rs_all[it][:]
                            )
                        nc.scalar.dma_start(
                            out_d.ap()[
                                b, 4 * ch : 4 * ch + 4, P * it : P * it + P, :
                            ].rearrange("c i w -> i c w"),
                            osb[:],
                        )

    nc.compile()
    _CACHE["nc"] = nc
    return nc


def _make_consts(Wq, bq, Wk, bk, Wv, bv):
    wqk = np.zeros((P, 32), np.float32)
    for g in range(G):
        for c in range(C):
            wqk[g * C + c, g] = Wq[0, c]
            wqk[g * C + c, 16 + g] = Wk[0, c]
    wv = np.zeros((P, P), np.float32)
    for g in range(G):
        for ci in range(C):
            for co in range(C):
                wv[g * C + ci, g * C + co] = Wv[co, ci]
    bqk = np.concatenate([np.full(16, bq[0]), np.full(16, bk[0])] * 4).astype(
        np.float32
    )[:, None]
    bvv = np.tile(bv.astype(np.float32), G)[:, None]
    import ml_dtypes

    eyef = np.eye(P, dtype=np.float32)
    eyeb = np.eye(P).astype(ml_dtypes.bfloat16)
    wqkh = wqk.astype(ml_dtypes.bfloat16)
    wqkl = (wqk - wqkh.astype(np.float32)).astype(ml_dtypes.bfloat16)
    return (wqkh, wqkl, wv.astype(ml_dtypes.bfloat16), bqk, bvv, eyef, eyeb)


def _split_x(x):
    import ml_dtypes

    x = np.asarray(x, dtype=np.float32)
    xh = x.astype(ml_dtypes.bfloat16)
    xl = (x - xh.astype(np.float32)).astype(ml_dtypes.bfloat16)
    # [B,C,H,W] -> [B, (g c)=128, r=NR, W]   (p = g*C + c, i = r*G + g)
    perm = lambda a: np.ascontiguousarray(
        a.reshape(B, C, NR, G, W).transpose(0, 3, 1, 2, 4).reshape(B, G * C, NR, W)
    )
    return perm(xh), perm(xl)


def kernel(x, Wq, bq, Wk, bk, Wv, bv):
    sys.path.insert(0, "/opt/trn_rl_repo")
    from concourse.bass_utils import run_bass_kernel_spmd

    nc = _build()
    wqkh, wqkl, wv, bqk, bvv, eyef, eyeb = _make_consts(
        np.asarray(Wq), np.asarray(bq), np.asarray(Wk), np.asarray(bk),
        np.asarray(Wv), np.asarray(bv),
    )
    xh, xl = _split_x(x)
    in_maps = []
    for core in range(NCORES):
        in_maps.append(
            {
                "xh": xh[BPC * core : BPC * core + BPC],
                "xl": xl[BPC * core : BPC * core + BPC],
                "wqkh": wqkh,
                "wqkl": wqkl,
                "wv": wv,
                "bqk": bqk,
                "bv": bvv,
                "identf": eyef,
                "identb": eyeb,
            }
        )
    res = run_bass_kernel_spmd(nc, in_maps, core_ids=list(range(NCORES)))
    out = np.concatenate([r["out"] for r in res.results], axis=0)
    return out


# revision 22
# speedup vs baseline: 1.7154x; 1.0183x over previous
"""Trainium2 Bass kernel for nn_Attention (B=16, C=8, H=W=512).

Per sample b:
  q = Wq.x + bq   [1,H,W]
  k = Wk.x + bk   [1,H,W]
  v = Wv.x + bv   [C,H,W]
  S[i,j] = sum_w q[i,w] k[j,w]; A = softmax_j(S); out[c,i,w] = sum_j A[i,j] v[c,j,w]

Sharding: data-parallel over batch, 2 samples per core, 8 cores, no collectives.

Per-core dataflow (per sample):
  - x loaded in "grouped" layout xg[r]: [128=(g,c), 512=w], g=16 rows, c=8 channels.
  - 1x1 convs as block-diagonal matmuls on TensorE (f32r):
      v-MM: lhsT=Wv_bd [128,128] -> psum grouped v; qk-MM: lhsT=Wqk_bd [128,32],
      4 col-group MMs fill a [128,512] psum = 64 rows of interleaved q/k.
  - v: psum -> SBUF (bias add + bf16 cast on ScalarE), then SBUF->SBUF DMA
      rearranges grouped v into channel-plane tiles vplane[jt]: [128=j, 8=c, 512=w].
  - q,k: psum -> SBUF (bias add), PE-transpose (f32r) -> qkT[wt]: [128=w, 1024=(q|k cols)].
  - S-MM (f32r): per i-tile accumulate 4 w-tiles; softmax via DVE reduce_max +
      ScalarE Exp(bias=-max, accum_out=rowsum) -> E (bf16, unnormalized).
  - A-transpose (bf16) -> AT[jt]; out-MM (bf16): per (i-tile, c) accumulate 4 j-tiles.
  - out: psum -> SBUF scaled by 1/rowsum (per-partition) on DVE, DMA out per i-tile.
"""

import os
import sys

import numpy as np

B, C, H, W = 16, 8, 512, 512
NCORES = 8
BPC = B // NCORES  # samples per core
P = 128
G = 16  # rows per group (P // C)
NR = H // G  # 32 row-groups per sample

_CACHE = {}


def _build():
    if "nc" in _CACHE:
        return _CACHE["nc"]
    sys.path.insert(0, "/opt/trn_rl_repo")
    import concourse.bass as bass
    import concourse.tile as tile
    from concourse import bacc, mybir

    f32 = mybir.dt.float32
    f32r = mybir.dt.float32r
    bf16 = mybir.dt.bfloat16
    AF = mybir.ActivationFunctionType
    AX = mybir.AxisListType

    nc = bacc.Bacc("TRN2", target_bir_lowering=False, debug=False)

    xh_d = nc.declare_dram_parameter("xh", [BPC, P, NR, W], bf16, isOutput=False)
    xl_d = nc.declare_dram_parameter("xl", [BPC, P, NR, W], bf16, isOutput=False)
    wqkh_d = nc.declare_dram_parameter("wqkh", [P, 32], bf16, isOutput=False)
    wqkl_d = nc.declare_dram_parameter("wqkl", [P, 32], bf16, isOutput=False)
    wv_d = nc.declare_dram_parameter("wv", [P, P], bf16, isOutput=False)
    bqk_d = nc.declare_dram_parameter("bqk", [P, 1], f32, isOutput=False)
    bv_d = nc.declare_dram_parameter("bv", [P, 1], f32, isOutput=False)
    idf_d = nc.declare_dram_parameter("identf", [P, P], f32, isOutput=False)
    idb_d = nc.declare_dram_parameter("identb", [P, P], bf16, isOutput=False)
    out_d = nc.declare_dram_parameter("out", [BPC, C, H, W], f32, isOutput=True)

    with tile.TileContext(nc) as tc:
        with (
            tc.tile_pool(name="consts", bufs=1) as consts,
            tc.tile_pool(name="xg", bufs=3) as xg_pool,
            tc.tile_pool(name="vg", bufs=2) as vg_pool,
            tc.tile_pool(name="vplane", bufs=2) as vp_pool,
            tc.tile_pool(name="qksb", bufs=10) as qk_pool,
            tc.tile_pool(name="qkt", bufs=2) as qkt_pool,
            tc.tile_pool(name="esb", bufs=8) as e_pool,
            tc.tile_pool(name="atsb", bufs=8) as at_pool,
            tc.tile_pool(name="osb", bufs=2) as o_pool,
            tc.tile_pool(name="stats", bufs=16) as st_pool,
            tc.tile_pool(name="ps", bufs=3, space="PSUM") as ps_pool,
            tc.tile_pool(name="ps_v", bufs=1, space="PSUM") as psv_pool,
            tc.tile_pool(name="ps_at", bufs=1, space="PSUM") as psat_pool,
        ):
            wqkh = consts.tile([P, 32], bf16)
            nc.sync.dma_start(wqkh[:], wqkh_d.ap())
            wqkl = consts.tile([P, 32], bf16)
            nc.sync.dma_start(wqkl[:], wqkl_d.ap())
            wv = consts.tile([P, P], bf16)
            nc.sync.dma_start(wv[:], wv_d.ap())
            bqk = consts.tile([P, 1], f32)
            nc.sync.dma_start(bqk[:], bqk_d.ap())
            bv = consts.tile([P, 1], f32)
            nc.sync.dma_start(bv[:], bv_d.ap())
            idf = consts.tile([P, P], f32)
            nc.sync.dma_start(idf[:], idf_d.ap())
            idb = consts.tile([P, P], bf16)
            nc.sync.dma_start(idb[:], idb_d.ap())

            def phase1(b):
                st = {}
                xbh = xh_d.ap()[b]  # [128, NR, W] grouped-partition-major
                xbl = xl_d.ap()[b]
                st["vplane"] = [
                    vp_pool.tile([P, C, W], bf16, name=f"vplane{i}") for i in range(4)
                ]
                st["qk_sb"] = []
                qk_psums = []
                RB = 8  # row-groups per x DMA (8KB contiguous runs)
                xqh = xql = psv = None
                for r in range(NR):
                    if r % RB == 0:
                        xqh = xg_pool.tile([P, RB, W], bf16, name="xqh")
                        nc.sync.dma_start(xqh[:], xbh[:, r : r + RB, :])
                        xql = xg_pool.tile([P, RB, W], bf16, name="xql")
                        nc.sync.dma_start(xql[:], xbl[:, r : r + RB, :])
                    xgh = xqh[:, r % RB, :]
                    xgl = xql[:, r % RB, :]
                    # v conv: grouped psum, batched 4 row-groups per 4-bank psum
                    if r % 4 == 0:
                        psv = psv_pool.tile([P, 4, W], f32, name="psv")
                    nc.tensor.matmul(
                        psv[:, r % 4, :], wv[:], xgh, start=True, stop=True
                    )
                    if r % 4 == 3:
                        vg = vg_pool.tile([P, 4, W], bf16)
                        nc.scalar.activation(vg[:], psv[:], AF.Identity, bias=bv[:])
                        # bridge: grouped -> channel-plane (SBUF->SBUF DMA)
                        for ri in range(4):
                            rr = r - 3 + ri
                            jt, sl = rr // 8, G * (rr % 8)
                            eng = nc.sync if ri % 2 == 0 else nc.scalar
                            eng.dma_start(
                                st["vplane"][jt][sl : sl + G, :, :], vg[:, ri, :]
                            )
                    # qk conv: 4 col-group MMs per [128,512] psum (64 rows)
                    if r % 4 == 0:
                        psqk = ps_pool.tile([P, W], f32, name="ps")
                        qk_psums.append(psqk)
                    m = r % 4
                    for wpart, xgpart, stt, sp in (
                        (wqkh, xgh, True, False),
                        (wqkh, xgl, False, False),
                        (wqkl, xgh, False, True),
                    ):
                        nc.tensor.matmul(
                            qk_psums[-1][32 * m : 32 * m + 32, :],
                            wpart[:],
                            xgpart,
                            start=stt,
                            stop=sp,
                            tile_position=(0, 32 * m),
                            skip_group_check=True,
                        )
                    if m == 3:
                        sb = qk_pool.tile([P, W], f32)
                        nc.scalar.activation(
                            sb[:], qk_psums[-1][:], AF.Identity, bias=bqk[:]
                        )
                        st["qk_sb"].append(sb)
                return st

            def phase2(b, st):
                # transpose q/k -> qkT[wt] [128=w, 1024=(q cols | k cols)]
                st["qkt"] = [
                    qkt_pool.tile([P, 2 * W], f32r, name=f"qkt{i}") for i in range(4)
                ]
                qkt = st["qkt"]
                for wt in range(4):
                    for th in range(2):
                        pst = ps_pool.tile([P, W], f32, name="ps")
                        for q in range(4):
                            sb = st["qk_sb"][4 * th + q]
                            nc.tensor.transpose(
                                pst[:, P * q : P * q + P],
                                sb[:, P * wt : P * wt + P],
                                idf[:],
                            )
                        # de-interleave [blk pair qk s] -> [qk (blk pair s)]
                        dst = qkt[wt][:].rearrange(
                            "p (qk half blk pair s) -> p qk half blk pair s",
                            qk=2, half=2, pair=4, s=G,
                        )[:, :, th]
                        csrc = pst[:].rearrange(
                            "p (blk pair qk s) -> p qk blk pair s",
                            qk=2, s=G, pair=4,
                        )
                        if th == 0:
                            nc.scalar.copy(dst, csrc)
                        else:
                            nc.vector.tensor_copy(dst, csrc)

            def phase3(b, st):
                # S matmul (f32r) + softmax
                st["e_sb"] = []
                st["rs"] = []
                qkt = st["qkt"]
                for it in range(4):
                    pss = ps_pool.tile([P, W], f32, name="ps")
                    qoff = P * it
                    for jh in range(2):
                        for wt in range(4):
                            nc.tensor.matmul(
                                pss[:, 256 * jh : 256 * jh + 256],
                                qkt[wt][:, qoff : qoff + P],
                                qkt[wt][:, W + 256 * jh : W + 256 * jh + 256],
                                start=(wt == 0),
                                stop=(wt == 3),
                                skip_group_check=True,
                            )
                    mx = st_pool.tile([P, 1], f32)
                    nc.vector.reduce_max(mx[:], pss[:], axis=AX.X, negate=True)
                    esb = e_pool.tile([P, W], bf16)
                    sm = st_pool.tile([P, 1], f32)
                    nc.scalar.activation(
                        esb[:], pss[:], AF.Exp, bias=mx[:], accum_out=sm[:]
                    )
                    rs = st_pool.tile([P, 1], f32)
                    nc.vector.reciprocal(rs[:], sm[:])
                    st["e_sb"].append(esb)
                    st["rs"].append(rs)

            def phase4(b, st):
                # transpose A (bf16) -> AT[jt]
                st["at"] = []
                for jt in range(4):
                    psa = psat_pool.tile([P, W], bf16, name="psa")
                    for it in range(4):
                        nc.tensor.transpose(
                            psa[:, P * it : P * it + P],
                            st["e_sb"][it][:, P * jt : P * jt + P],
                            idb[:],
                        )
                    atsb = at_pool.tile([P, W], bf16)
                    nc.vector.tensor_copy(atsb[:], psa[:])
                    st["at"].append(atsb)

            def phase5(b, st, its):
                # out matmul + normalize + store
                for it in its:
                    for ch in range(2):
                        osb = o_pool.tile([P, 4, W], f32)
                        for cc in range(4):
                            c = 4 * ch + cc
                            pso = ps_pool.tile([P, W], f32, name="ps")
                            for jt in range(4):
                                nc.tensor.matmul(
                                    pso[:],
                                    st["at"][jt][:, P * it : P * it + P],
                                    st["vplane"][jt][:, c, :],
                                    start=(jt == 0),
                                    stop=(jt == 3),
                                )
                            nc.vector.tensor_scalar_mul(
                                osb[:, cc, :], pso[:], st["rs"][it][:]
                            )
                        nc.scalar.dma_start(
                            out_d.ap()[
                                b, 4 * ch : 4 * ch + 4, P * it : P * it + P, :
                            ].rearrange("c i w -> i c w"),
                            osb[:],
                        )

            # pipelined emission: sample 1's conv stream fills sample 0's
            # softmax bubbles; sample 0's out-matmuls fill sample 1's.
            s0 = phase1(0)
            phase2(0, s0)
            phase3(0, s0)
            phase4(0, s0)
            phase5(0, s0, [0, 1])
            s1 = phase1(1)
            phase5(0, s0, [2, 3])
            phase2(1, s1)
            phase3(1, s1)
            phase4(1, s1)
            phase5(1, s1, [0, 1, 2, 3])

    nc.compile()
    _CACHE["nc"] = nc
    return nc


def _make_consts(Wq, bq, Wk, bk, Wv, bv):
    wqk = np.zeros((P, 32), np.float32)
    for g in range(G):
        for c in range(C):
            wqk[g * C + c, g] = Wq[0, c]
            wqk[g * C + c, 16 + g] = Wk[0, c]
    wv = np.zeros((P, P), np.float32)
    for g in range(G):
        for ci in range(C):
            for co in range(C):
                wv[g * C + ci, g * C + co] = Wv[co, ci]
    bqk = np.concatenate([np.full(16, bq[0]), np.full(16, bk[0])] * 4).astype(
        np.float32
    )[:, None]
    bvv = np.tile(bv.astype(np.float32), G)[:, None]
    import ml_dtypes

    eyef = np.eye(P, dtype=np.float32)
    eyeb = np.eye(P).astype(ml_dtypes.bfloat16)
    wqkh = wqk.astype(ml_dtypes.bfloat16)
    wqkl = (wqk - wqkh.astype(np.float32)).astype(ml_dtypes.bfloat16)
    return (wqkh, wqkl, wv.astype(ml_dtypes.bfloat16), bqk, bvv, eyef, eyeb)


def _split_x(x):
    import ml_dtypes

    x = np.asarray(x, dtype=np.float32)
    xh = x.astype(ml_dtypes.bfloat16)
    xl = (x - xh.astype(np.float32)).astype(ml_dtypes.bfloat16)
    # [B,C,H,W] -> [B, (g c)=128, r=NR, W]   (p = g*C + c, i = r*G + g)
    perm = lambda a: np.ascontiguousarray(
        a.reshape(B, C, NR, G, W).transpose(0, 3, 1, 2, 4).reshape(B, G * C, NR, W)
    )
    return perm(xh), perm(xl)


def kernel(x, Wq, bq, Wk, bk, Wv, bv):
    sys.path.insert(0, "/opt/trn_rl_repo")
    from concourse.bass_utils import run_bass_kernel_spmd

    nc = _build()
    wqkh, wqkl, wv, bqk, bvv, eyef, eyeb = _make_consts(
        np.asarray(Wq), np.asarray(bq), np.asarray(Wk), np.asarray(bk),
        np.asarray(Wv), np.asarray(bv),
    )
    xh, xl = _split_x(x)
    in_maps = []
    for core in range(NCORES):
        in_maps.append(
            {
                "xh": xh[BPC * core : BPC * core + BPC],
                "xl": xl[BPC * core : BPC * core + BPC],
                "wqkh": wqkh,
                "wqkl": wqkl,
                "wv": wv,
                "bqk": bqk,
                "bv": bvv,
                "identf": eyef,
                "identb": eyeb,
            }
        )
    res = run_bass_kernel_spmd(nc, in_maps, core_ids=list(range(NCORES)))
    out = np.concatenate([r["out"] for r in res.results], axis=0)
    return out
